# revision 1
# baseline (speedup 1.0000x reference)
"""Fused OOQKV attention-with-generated-transform kernel for Trainium2.

Math (per head h):
  g = gelu(x @ Wg_h + bg_h)            # [T, 64, 64] per-token transform
  q,k,v = x @ W{q,k,v}_h + b           # [T, 64]
  qg[t] = q[t] @ g[t]
  att = softmax(qg @ k^T)              # per batch, no scaling
  out_h = att @ v

Sharding: head-parallel, 1 head per core (8 heads, 8 cores); every core
reads the full (host-pre-transposed) xT.

Per-core schedule:
  phase 1 (per 128-token tile): fused q|v_aug|k projection and the
    32768-wide g projection, grouped so consecutive PE matmuls share the
    stationary xT slice (f32r weight switches cost ~2x); biases are K=1
    bf16 matmuls (bf16 keeps them at stream rate; bias magnitudes are
    ~0.04 so bf16 rounding is ~1e-4 absolute). ACT applies exact gelu,
    writing each 512-chunk transposed to (e-major, d-minor) layout so the
    DVE qg contraction multiplies contiguously against a broadcast q view
    and reduces over a contiguous innermost d. PE transposes build kT and
    qgT for phase 2.
  phase 2 (per batch, per 512 query cols): S^T = kT-slice.T @ qgT on PE,
    exp on ACT (no max subtraction; |scores| < 70 so fp32 exp is exact
    enough), then out^T accumulated over m-tiles with v augmented by a
    ones column so row 64 carries the softmax denominator.
Host divides by the denominator row and transposes during the gather.

Matmuls run in float32r (fp32-reduced: 1 cycle/row streaming, ~1e-4
matmul rel err measured on HW); end-to-end rel err vs the fp32 reference
is ~1e-3.
"""

import sys

sys.path.insert(0, "/opt/trn_rl_repo")

import numpy as np

B, N, E, H, D = 4, 1024, 512, 8, 64
T = B * N                 # 4096 flattened tokens
OC = 512                  # g-matmul output chunk
NOC = (D * D) // OC       # 8 chunks per head
DPC = OC // D             # 8 d-values per chunk
NTT = T // 128            # 32 token tiles
NKT = E // 128            # 4 contraction tiles
QVKW = 256                # fused q|v_aug|k projection width (zero padded)
M = 8                     # cores

_cache = {}


def _build():
    if "nc" in _cache:
        return _cache["nc"]
    from contextlib import ExitStack

    import concourse.bass as bass
    import concourse.bacc as bacc
    import concourse.mybir as mybir
    import concourse.tile as tile
    from concourse.masks import make_identity

    F32 = mybir.dt.float32
    F32R = mybir.dt.float32r
    BF16 = mybir.dt.bfloat16
    AF = mybir.ActivationFunctionType
    ALU = mybir.AluOpType
    AX = mybir.AxisListType

    nc = bacc.Bacc(trn_type="TRN2")
    xT_d = nc.dram_tensor("xT", [E, T], F32R, kind="ExternalInput")
    Wg_d = nc.dram_tensor("Wg", [E, D * D], F32R, kind="ExternalInput")
    bg_d = nc.dram_tensor("bg", [1, D * D], BF16, kind="ExternalInput")
    Wqvk_d = nc.dram_tensor("Wqvk", [E, QVKW], F32R, kind="ExternalInput")
    bqvk_d = nc.dram_tensor("bqvk", [1, QVKW], BF16, kind="ExternalInput")
    outT_d = nc.dram_tensor("outT", [D + 1, T], F32, kind="ExternalOutput")

    with tile.TileContext(nc) as tc, ExitStack() as ctx:
        const = ctx.enter_context(tc.tile_pool(name="const", bufs=1))
        acts = ctx.enter_context(tc.tile_pool(name="acts", bufs=1))

        wqvk_sb = []
        for kt in range(NKT):
            wqt = const.tile([128, QVKW], F32R, tag=f"wqvk{kt}")
            nc.sync.dma_start(wqt[:], Wqvk_d[kt * 128:(kt + 1) * 128, :])
            wqvk_sb.append(wqt)
        bg_sb = const.tile([1, D * D], BF16)
        nc.sync.dma_start(bg_sb[:], bg_d[:, :])
        bqvk_sb = const.tile([1, QVKW], BF16)
        nc.sync.dma_start(bqvk_sb[:], bqvk_d[:, :])
        ones32 = const.tile([1, 128], F32)
        nc.gpsimd.memset(ones32[:], 1.0)
        ones_b = const.tile([1, 128], BF16)
        nc.gpsimd.tensor_copy(ones_b[:], ones32[:])
        ident = const.tile([128, 128], F32)
        make_identity(nc, ident[:])

        # persistent per-head activations
        q_sb = acts.tile([128, NTT, D], F32)       # q, natural layout
        v_sb = acts.tile([128, NTT, D + 1], F32R)  # v | ones column
        kT_sb = acts.tile([D, T], F32R)
        qgT_sb = acts.tile([D, T], F32R)

        # ---------------- phase 1: projections, g, qg ----------------
        with ExitStack() as p1:
            xpool = p1.enter_context(tc.tile_pool(name="xp", bufs=2))
            wgpool = p1.enter_context(tc.tile_pool(name="wgp", bufs=1))
            wg_sb = []
            for kt in range(NKT):
                wgt = wgpool.tile([128, D * D], F32R, tag=f"wg{kt}",
                                  name=f"wg{kt}")
                wg_sb.append(wgt)
            QL = (D * D) // 4
            for quar in range(4):
                for kt in range(NKT):
                    nc.scalar.dma_start(
                        wg_sb[kt][:, quar * QL:(quar + 1) * QL],
                        Wg_d[kt * 128:(kt + 1) * 128,
                             quar * QL:(quar + 1) * QL])
            gpool = p1.enter_context(tc.tile_pool(name="gp", bufs=5))
            dpool = p1.enter_context(tc.tile_pool(name="dp", bufs=4))
            pp_g = p1.enter_context(
                tc.tile_pool(name="pg", bufs=7, space="PSUM"))
            pp_qvk = pp_g
            pp_tr = p1.enter_context(
                tc.tile_pool(name="ptr", bufs=1, space="PSUM"))

            pending = []  # (tc0, k_nat, qg_t) awaiting PE transpose

            def flush_pending():
                for ptc0, pk, pqg in pending:
                    ptr = pp_tr.tile([D, 128], F32, tag="tr", name="ktr")
                    nc.tensor.transpose(ptr[:], pk[:], ident[:])
                    nc.vector.tensor_copy(kT_sb[:, ptc0:ptc0 + 128], ptr[:])
                    ptr2 = pp_tr.tile([D, 128], F32, tag="tr", name="qgtr")
                    nc.tensor.transpose(ptr2[:], pqg[:], ident[:])
                    nc.vector.tensor_copy(qgT_sb[:, ptc0:ptc0 + 128], ptr2[:])
                pending.clear()

            for tt in range(NTT):
                tc0 = tt * 128
                xs = []
                for kt in range(NKT):
                    xt = xpool.tile([128, 128], F32R, tag=f"x{kt}")
                    nc.sync.dma_start(
                        xt[:], xT_d[kt * 128:(kt + 1) * 128, tc0:tc0 + 128])
                    xs.append(xt)

                # two rounds of 4 g-chunks; round 0 also carries the qvk
                # projection so each (round, kt) is a same-lhsT matmul run
                pq = pp_qvk.tile([128, OC], F32, tag="pg", name="pq")
                pgs = {}
                for rnd in range(2):
                    for kt in range(NKT):
                        if rnd == 0:
                            nc.tensor.matmul(pq[:, 0:QVKW], xs[kt][:],
                                             wqvk_sb[kt][:],
                                             start=(kt == 0), stop=False)
                        for oc in range(rnd * 4, rnd * 4 + 4):
                            oc0 = oc * OC
                            if kt == 0:
                                pgs[oc] = pp_g.tile([128, OC], F32, tag="pg", name=f"pg{oc}")
                            nc.tensor.matmul(
                                pgs[oc][:], xs[kt][:],
                                wg_sb[kt][:, oc0:oc0 + OC],
                                start=(kt == 0), stop=False)
                    if rnd == 0:
                        nc.tensor.matmul(pq[:, 0:QVKW], ones_b[:],
                                         bqvk_sb[:], start=False, stop=True)
                    for oc in range(rnd * 4, rnd * 4 + 4):
                        oc0 = oc * OC
                        nc.tensor.matmul(pgs[oc][:], ones_b[:],
                                         bg_sb[:, oc0:oc0 + OC],
                                         start=False, stop=True)
                    if rnd == 0:
                        flush_pending()  # prior tile's transposes mid-stream

                nc.vector.tensor_copy(q_sb[:, tt, :], pq[:, 0:D])
                nc.vector.tensor_copy(v_sb[:, tt, :], pq[:, D:2 * D + 1])
                k_nat = dpool.tile([128, D], F32, tag="knat")
                nc.vector.tensor_copy(k_nat[:], pq[:, 2 * D + 1:3 * D + 1])

                # gelu + qg contraction per chunk
                qg_part = dpool.tile([128, NOC, D], F32, tag="qgp")
                for oc in range(NOC):
                    # gelu, written (e-major, d-minor) so the d-reduce is
                    # contiguous
                    g_t = gpool.tile([128, OC], F32, tag="g")
                    gw = g_t[:]
                    g_ed = bass.AP(tensor=gw.tensor, offset=gw.offset,
                                   ap=[gw.ap[0], [1, DPC], [DPC, D]])
                    nc.scalar.activation(g_ed, pgs[oc][:], AF.Gelu)
                    prod = gpool.tile([128, OC], F32, tag="prod")
                    qs = q_sb[:, tt, :]
                    q3 = bass.AP(
                        tensor=qs.tensor,
                        offset=qs.offset + oc * DPC,
                        ap=[qs.ap[0], [0, D], [1, DPC]])
                    nc.vector.tensor_tensor(
                        prod[:].rearrange("p (e d) -> p e d", d=DPC),
                        g_t[:].rearrange("p (e d) -> p e d", d=DPC),
                        q3, op=ALU.mult)
                    nc.vector.tensor_reduce(
                        qg_part[:, oc, :],
                        prod[:].rearrange("p (e d) -> p e d", d=DPC),
                        axis=AX.X, op=ALU.add)
                qg_t = dpool.tile([128, D], F32, tag="qg")
                qp = qg_part[:]
                qpv = bass.AP(tensor=qp.tensor, offset=qp.offset,
                              ap=[qp.ap[0], [1, D], [D, NOC]])
                nc.vector.tensor_reduce(qg_t[:], qpv, axis=AX.X, op=ALU.add)
                pending.append((tc0, k_nat, qg_t))
            flush_pending()

        # ---------------- phase 2: attention ----------------
        with ExitStack() as p2:
            espool = p2.enter_context(tc.tile_pool(name="es", bufs=34))
            outp = p2.enter_context(tc.tile_pool(name="outp", bufs=4))
            pp_s = p2.enter_context(
                tc.tile_pool(name="psc", bufs=6, space="PSUM"))
            pp_av = p2.enter_context(
                tc.tile_pool(name="pav", bufs=2, space="PSUM"))

            NMT = N // 128  # m tiles per batch
            NNC = N // OC   # n chunks per batch
            pending_av = []  # (b, nch, es-dict) awaiting av emission

            def emit_av():
                if not pending_av:
                    return
                by_b = {}
                for bb, nch, esd in pending_av:
                    by_b.setdefault(bb, {})[nch] = esd
                for bb, chunks in by_b.items():
                    pavs = {nch: pp_av.tile([D + 1, OC], F32, tag="av",
                                            name=f"pav{nch}")
                            for nch in chunks}
                    for mt in range(NMT):
                        for nch, esd in chunks.items():
                            nc.tensor.matmul(pavs[nch][:],
                                             v_sb[:, bb * NMT + mt, :],
                                             esd[mt][:],
                                             start=(mt == 0),
                                             stop=(mt == NMT - 1))
                    for nch in chunks:
                        nc0 = bb * N + nch * OC
                        o_t = outp.tile([D + 1, OC], F32, tag="o", name="o_t")
                        nc.vector.tensor_copy(o_t[:], pavs[nch][:])
                        nc.sync.dma_start(outT_d[:, nc0:nc0 + OC], o_t[:])
                pending_av.clear()

            for b in range(B):
                es = {}
                # S^T and exp for the whole batch; kT slice (lhsT) is
                # reused across both n-chunks
                for mt in range(NMT):
                    if mt == 2:
                        emit_av()  # prior batch's av, mid-stream
                    mc0 = b * N + mt * 128
                    for nch in range(NNC):
                        nc0 = b * N + nch * OC
                        ps_ = pp_s.tile([128, OC], F32, tag="s")
                        nc.tensor.matmul(ps_[:], kT_sb[:, mc0:mc0 + 128],
                                         qgT_sb[:, nc0:nc0 + OC],
                                         start=True, stop=True)
                        e_t = espool.tile([128, OC], F32R, tag="es")
                        nc.scalar.activation(e_t[:], ps_[:], AF.Exp)
                        es[(mt, nch)] = e_t
                for nch in range(NNC):
                    pending_av.append((b, nch, {mt: es[(mt, nch)]
                                                for mt in range(NMT)}))
            emit_av()

    nc.compile()
    _cache["nc"] = nc
    return nc


def _make_in_maps(x, Wq, bq, Wk, bk, Wv, bv, Wg, bg):
    import ml_dtypes
    x = np.asarray(x, dtype=np.float32)
    xT = np.ascontiguousarray(x.reshape(T, E).T)
    in_maps = []
    for h in range(M):
        c0 = h * D
        Wqvk = np.zeros((E, QVKW), dtype=np.float32)
        Wqvk[:, 0:D] = Wq[:, c0:c0 + D]
        Wqvk[:, D:2 * D] = Wv[:, c0:c0 + D]
        # column 2*D is the ones column of v_aug: weight 0, bias 1
        Wqvk[:, 2 * D + 1:3 * D + 1] = Wk[:, c0:c0 + D]
        bqvk = np.zeros((1, QVKW), dtype=np.float32)
        bqvk[0, 0:D] = bq[c0:c0 + D]
        bqvk[0, D:2 * D] = bv[c0:c0 + D]
        bqvk[0, 2 * D] = 1.0
        bqvk[0, 2 * D + 1:3 * D + 1] = bk[c0:c0 + D]
        g0 = h * D * D
        in_maps.append(dict(
            xT=xT,
            Wg=np.ascontiguousarray(Wg[:, g0:g0 + D * D], dtype=np.float32),
            bg=np.ascontiguousarray(bg[g0:g0 + D * D], dtype=np.float32)
            .reshape(1, D * D).astype(ml_dtypes.bfloat16),
            Wqvk=Wqvk,
            bqvk=bqvk.astype(ml_dtypes.bfloat16),
        ))
    return in_maps


def kernel(x, Wq, bq, Wk, bk, Wv, bv, Wg, bg):
    from concourse import bass_utils

    nc = _build()
    in_maps = _make_in_maps(x, Wq, bq, Wk, bk, Wv, bv, Wg, bg)
    res = bass_utils.run_bass_kernel_spmd(nc, in_maps, core_ids=list(range(M)))
    out = np.empty((B, N, H, D), dtype=np.float32)
    for h in range(M):
        oT = res.results[h]["outT"]           # [65, T]
        o = (oT[:D] / oT[D:D + 1]).T          # [T, 64]
        out[:, :, h, :] = o.reshape(B, N, D)
    return out.reshape(B, N, E)

